# revision 7
# baseline (speedup 1.0000x reference)
"""Multi-head attention (B=2, L=4096, D=512, H=8, HD=64) on 8 trn2 NeuronCores.

Sharding: data-parallel over batch (2) x tensor-parallel over head-pairs (4):
core c handles batch c//4, heads (c%4)*2 and (c%4)*2+1. Each core projects
Q/K/V for its two heads (columns of Wq/Wk/Wv), runs flash-style attention
(S^T orientation, no-max-subtraction exp since logits are small, denominator
via an appended ones-column in V), applies its rows of Wo, and returns a
partial [L, D] output. Host sums the 4 partials per batch and adds bo.

All matmul operands use float32r (rounded fp32): full PE speed at N>=512
with ~1e-4 matmul precision. PSUM accumulation is fp32.
"""

import sys
import types

import numpy as np

B, L, D = 2, 4096, 512
H, HD = 8, 64
NCORES = 8
HPC = 2          # heads per core
HD2 = HPC * HD   # 128
QB = 512         # query block (free dim of S^T tiles per head)
NQB = L // QB    # 8
KC = 128         # key-position chunk (partition dim of S^T tiles)
NKC = L // KC    # 32
NDC = D // 128   # 4 contraction chunks for projections

_CACHED_NC = None


def _ensure_axon_hook():
    """Register the NTFF profile hook boot() couldn't (stub antenv lacks
    axon_hooks). Harmless when tracing is never requested."""
    try:
        from antenv.axon_hooks import get_axon_ntff_profile_hook  # noqa: F401
        return
    except ImportError:
        pass
    hook = None
    try:
        from trn_agent_boot.trn_boot import _ntff_profile_via_ctypes
        hook = _ntff_profile_via_ctypes("/opt/axon/libaxon_pjrt.so")
    except Exception:
        pass
    mod = types.ModuleType("antenv.axon_hooks")
    mod.get_axon_ntff_profile_hook = lambda: hook
    mod.set_axon_ntff_profile_hook = lambda h: None
    sys.modules["antenv.axon_hooks"] = mod


def _build_nc():
    from concourse import bacc
    import concourse.mybir as mybir
    import concourse.tile as tile

    f32 = mybir.dt.float32
    f32r = mybir.dt.float32r
    AF = mybir.ActivationFunctionType

    nc = bacc.Bacc("TRN2", target_bir_lowering=False, debug=False,
                   num_devices=NCORES)

    xq = nc.dram_tensor("xq", [D, L], f32, kind="ExternalInput")
    xk = nc.dram_tensor("xk", [D, L], f32, kind="ExternalInput")
    xv = nc.dram_tensor("xv", [D, L], f32, kind="ExternalInput")
    wq = nc.dram_tensor("wq", [D, HD2], f32, kind="ExternalInput")
    wk = nc.dram_tensor("wk", [D, HD2], f32, kind="ExternalInput")
    wv = nc.dram_tensor("wv", [D, HD2], f32, kind="ExternalInput")
    wo0 = nc.dram_tensor("wo0", [HD, D], f32, kind="ExternalInput")
    wo1 = nc.dram_tensor("wo1", [HD, D], f32, kind="ExternalInput")
    bq = nc.dram_tensor("bq", [HD2, 1], f32, kind="ExternalInput")
    bk = nc.dram_tensor("bk", [HD2, 1], f32, kind="ExternalInput")
    bvb = nc.dram_tensor("bvb", [128, HD2], f32, kind="ExternalInput")
    mb = nc.dram_tensor("mb", [KC, NKC], f32, kind="ExternalInput")
    out = nc.dram_tensor("out", [L, D], f32, kind="ExternalOutput")

    with tile.TileContext(nc) as tc:
        with (
            tc.tile_pool(name="singles", bufs=1) as singles,
            tc.tile_pool(name="xload", bufs=6) as xload,
            tc.tile_pool(name="xcast", bufs=6) as xcast,
            tc.tile_pool(name="qtp", bufs=NQB) as qtp,
            tc.tile_pool(name="ptp", bufs=4) as ptp,
            tc.tile_pool(name="xtp", bufs=4) as xtp,
            tc.tile_pool(name="op", bufs=3) as op,
            tc.tile_pool(name="small", bufs=4) as small,
            tc.tile_pool(name="dscr", bufs=2, space="DRAM") as dscr,
            tc.tile_pool(name="ps_s", bufs=2, space="PSUM") as ps_sp,
            tc.tile_pool(name="ps_u", bufs=2, space="PSUM") as ps_up,
            tc.tile_pool(name="ps_m", bufs=2, space="PSUM") as ps_mp,
        ):
            # ---------------- constants / weights ----------------
            def load_w(name, dram):
                wf = singles.tile([128, NDC, HD2], f32, tag=name + "f")
                nc.sync.dma_start(wf[:], dram.rearrange("(c p) m -> p c m", p=128))
                wr = singles.tile([128, NDC, HD2], f32r, tag=name)
                nc.vector.tensor_copy(wr[:], wf[:])
                return wr

            wq_sb = load_w("wq", wq)
            wk_sb = load_w("wk", wk)
            wv_sb = load_w("wv", wv)

            wo0_f = singles.tile([HD, D], f32, tag="wo0f")
            wo1_f = singles.tile([HD, D], f32, tag="wo1f")
            nc.sync.dma_start(wo0_f[:], wo0[:, :])
            nc.sync.dma_start(wo1_f[:], wo1[:, :])
            wo0_sb = singles.tile([HD, D], f32r, tag="wo0")
            wo1_sb = singles.tile([HD, D], f32r, tag="wo1")
            nc.vector.tensor_copy(wo0_sb[:], wo0_f[:])
            nc.vector.tensor_copy(wo1_sb[:], wo1_f[:])

            bq_sb = singles.tile([HD2, 1], f32, tag="bq")
            bk_sb = singles.tile([HD2, 1], f32, tag="bk")
            bvb_sb = singles.tile([128, HD2], f32, tag="bvb")
            mb_sb = singles.tile([KC, NKC], f32, tag="mb")
            nc.sync.dma_start(bq_sb[:], bq[:, :])
            nc.sync.dma_start(bk_sb[:], bk[:, :])
            nc.sync.dma_start(bvb_sb[:], bvb[:, :])
            nc.sync.dma_start(mb_sb[:], mb[:, :])

            # K^T [hd2, L] and V' [kpos, chunk, hd+1] per head, all f32r
            kt = singles.tile([HD2, L], f32r, tag="kt")
            v0 = singles.tile([128, NKC, HD + 1], f32r, tag="v0")
            v1 = singles.tile([128, NKC, HD + 1], f32r, tag="v1")
            nc.vector.memset(v0[:, :, HD:HD + 1].bitcast(f32), 1.0)
            nc.vector.memset(v1[:, :, HD:HD + 1].bitcast(f32), 1.0)

            def load_x_block(dram, lb):
                """DMA a [128, QB] f32 tile per D-chunk and round to f32r."""
                tiles = []
                for dc in range(NDC):
                    xf = xload.tile([128, QB], f32, tag="xl")
                    nc.sync.dma_start(
                        xf[:], dram[dc * 128:(dc + 1) * 128, lb * QB:(lb + 1) * QB])
                    xr = xcast.tile([128, QB], f32r, tag="xc")
                    nc.vector.tensor_copy(xr[:], xf[:])
                    tiles.append(xr)
                return tiles

            # ---------------- K projection ----------------
            for lb in range(NQB):
                xts = load_x_block(xk, lb)
                ps_kt = ps_mp.tile([128, QB], f32, tag="psm")
                for dc in range(NDC):
                    nc.tensor.matmul(ps_kt[:], wk_sb[:, dc, :], xts[dc][:],
                                     start=(dc == 0), stop=(dc == NDC - 1))
                nc.vector.tensor_scalar_add(
                    kt[:, lb * QB:(lb + 1) * QB], in0=ps_kt[:], scalar1=bk_sb[:])

            # ---------------- V projection ----------------
            for lb in range(NQB):
                xts = load_x_block(xv, lb)
                for j in range(4):
                    lc = lb * 4 + j
                    ps_v = ps_mp.tile([128, HD2], f32, tag="psm")
                    for dc in range(NDC):
                        nc.tensor.matmul(
                            ps_v[:], xts[dc][:, j * 128:(j + 1) * 128],
                            wv_sb[:, dc, :],
                            start=(dc == 0), stop=(dc == NDC - 1))
                    nc.vector.tensor_add(v0[:, lc, 0:HD], ps_v[:, 0:HD],
                                         bvb_sb[:, 0:HD])
                    nc.vector.tensor_add(v1[:, lc, 0:HD], ps_v[:, HD:HD2],
                                         bvb_sb[:, HD:HD2])

            # ---------------- attention + output, pipelined per q-block ----
            pending = None  # (u0, u1, qb) awaiting normalize + Wo

            def emit_norm_wo(u0, u1, qb):
                r0 = small.tile([HD + 1, QB], f32, tag="r0")
                r1 = small.tile([HD + 1, QB], f32, tag="r1")
                nc.vector.reciprocal(r0[HD:HD + 1, :], u0[HD:HD + 1, :])
                nc.vector.reciprocal(r1[HD:HD + 1, :], u1[HD:HD + 1, :])
                rb0 = small.tile([HD, QB], f32, tag="rb0")
                rb1 = small.tile([HD, QB], f32, tag="rb1")
                scr = dscr.tile([2, QB], f32, tag="scr")
                nc.sync.dma_start(scr[0:1, :], r0[HD:HD + 1, :])
                nc.sync.dma_start(scr[1:2, :], r1[HD:HD + 1, :])
                nc.sync.dma_start(rb0[:], scr[0:1, :].to_broadcast([HD, QB]))
                nc.sync.dma_start(rb1[:], scr[1:2, :].to_broadcast([HD, QB]))
                xt0 = xtp.tile([HD, QB], f32r, tag="xt0")
                xt1 = xtp.tile([HD, QB], f32r, tag="xt1")
                nc.vector.tensor_mul(xt0[:], u0[0:HD, :], rb0[:])
                nc.vector.tensor_mul(xt1[:], u1[0:HD, :], rb1[:])
                for j in range(4):
                    qs = slice(j * 128, (j + 1) * 128)
                    ps_o = ps_mp.tile([128, D], f32, tag="psm")
                    nc.tensor.matmul(ps_o[:], xt0[:, qs], wo0_sb[:],
                                     start=True, stop=False)
                    nc.tensor.matmul(ps_o[:], xt1[:, qs], wo1_sb[:],
                                     start=False, stop=True)
                    o_t = op.tile([128, D], f32, tag="ot")
                    nc.vector.tensor_copy(o_t[:], ps_o[:])
                    nc.sync.dma_start(
                        out[qb * QB + j * 128: qb * QB + (j + 1) * 128, :], o_t[:])

            for qb in range(NQB):
                # Q projection for this q-block -> qt [hd2, QB] f32r
                xts = load_x_block(xq, qb)
                ps_q = ps_mp.tile([128, QB], f32, tag="psm")
                for dc in range(NDC):
                    nc.tensor.matmul(ps_q[:], wq_sb[:, dc, :], xts[dc][:],
                                     start=(dc == 0), stop=(dc == NDC - 1))
                qt = qtp.tile([HD2, QB], f32r, tag="qt")
                nc.vector.tensor_scalar_add(qt[:], in0=ps_q[:], scalar1=bq_sb[:])

                u0 = ps_up.tile([HD + 1, QB], f32, tag="u")
                u1 = ps_up.tile([HD + 1, QB], f32, tag="u")

                def emit_pv(pt, c):
                    nc.tensor.matmul(u0[:], v0[:, c, :], pt[:, 0:QB],
                                     start=(c == 0), stop=(c == NKC - 1))
                    nc.tensor.matmul(u1[:], v1[:, c, :], pt[:, QB:2 * QB],
                                     start=(c == 0), stop=(c == NKC - 1))

                pv_q = []
                for c in range(NKC):
                    ks = slice(c * KC, (c + 1) * KC)
                    ps_s = ps_sp.tile([128, 2 * QB], f32, tag="pss")
                    nc.tensor.matmul(ps_s[:, 0:QB], kt[0:HD, ks], qt[0:HD, :],
                                     start=True, stop=True)
                    nc.tensor.matmul(ps_s[:, QB:2 * QB], kt[HD:HD2, ks],
                                     qt[HD:HD2, :], start=True, stop=True)
                    pt = ptp.tile([128, 2 * QB], f32r, tag="pt")
                    nc.scalar.activation(pt[:], ps_s[:], AF.Exp,
                                         bias=mb_sb[:, c:c + 1], scale=0.125)
                    pv_q.append((pt, c))
                    if len(pv_q) > 2:
                        emit_pv(*pv_q.pop(0))
                    if c == 3 and pending is not None:
                        emit_norm_wo(*pending)
                        pending = None
                for item in pv_q:
                    emit_pv(*item)
                pending = (u0, u1, qb)

            emit_norm_wo(*pending)

    nc.compile()
    return nc


def _get_nc():
    global _CACHED_NC
    if _CACHED_NC is None:
        _ensure_axon_hook()
        _CACHED_NC = _build_nc()
    return _CACHED_NC


def kernel(query, key, value, mask, Wq, bq, Wk, bk, Wv, bv, Wo, bo,
           _trace=False, _results_sink=None):
    from concourse.bass_utils import run_bass_kernel_spmd

    query = np.asarray(query, np.float32)
    key = np.asarray(key, np.float32)
    value = np.asarray(value, np.float32)
    mask = np.asarray(mask)
    Wq = np.asarray(Wq, np.float32)
    bq = np.asarray(bq, np.float32)
    Wk = np.asarray(Wk, np.float32)
    bk = np.asarray(bk, np.float32)
    Wv = np.asarray(Wv, np.float32)
    bv = np.asarray(bv, np.float32)
    Wo = np.asarray(Wo, np.float32)
    bo = np.asarray(bo, np.float32)

    nc = _get_nc()

    xqT = [np.ascontiguousarray(query[b].T) for b in range(B)]
    xkT = [np.ascontiguousarray(key[b].T) for b in range(B)]
    xvT = [np.ascontiguousarray(value[b].T) for b in range(B)]
    mbias = [
        np.ascontiguousarray(
            ((1 - mask[b].astype(np.float32)) * -1e30)
            .astype(np.float32).reshape(NKC, KC).T)
        for b in range(B)
    ]

    in_maps = []
    for core in range(NCORES):
        b = core // 4
        h0 = (core % 4) * HPC
        sl = slice(h0 * HD, (h0 + HPC) * HD)
        in_maps.append({
            "xq": xqT[b],
            "xk": xkT[b],
            "xv": xvT[b],
            "wq": np.ascontiguousarray(Wq[:, sl]),
            "wk": np.ascontiguousarray(Wk[:, sl]),
            "wv": np.ascontiguousarray(Wv[:, sl]),
            "wo0": np.ascontiguousarray(Wo[sl, :][0:HD]),
            "wo1": np.ascontiguousarray(Wo[sl, :][HD:HD2]),
            "bq": np.ascontiguousarray(bq[sl].reshape(HD2, 1)),
            "bk": np.ascontiguousarray(bk[sl].reshape(HD2, 1)),
            "bvb": np.ascontiguousarray(np.tile(bv[sl][None, :], (128, 1))),
            "mb": mbias[b],
        })

    res = run_bass_kernel_spmd(nc, in_maps, core_ids=list(range(NCORES)),
                               trace=_trace)
    if _results_sink is not None:
        _results_sink.append(res)

    final = np.empty((B, L, D), np.float32)
    for b in range(B):
        acc = res.results[4 * b]["out"].astype(np.float32).copy()
        for i in range(1, 4):
            acc += res.results[4 * b + i]["out"]
        final[b] = acc + bo[None, :]
    return final


# revision 8
# speedup vs baseline: 1.1537x; 1.1537x over previous
"""Multi-head attention (B=2, L=4096, D=512, H=8, HD=64) on 8 trn2 NeuronCores.

Sharding: data-parallel over batch (2) x tensor-parallel over head-pairs (4):
core c handles batch c//4, heads (c%4)*2 and (c%4)*2+1. Each core projects
Q/K/V for its two heads (columns of Wq/Wk/Wv), runs flash-style attention
(S^T orientation, no-max-subtraction exp since logits are small, denominator
via an appended ones-column in V), applies its rows of Wo, and returns a
partial [L, D] output. Host sums the 4 partials per batch and adds bo.

Precision: S-path matmuls (projections + scores) use bf16 operands (weight
loads hide via FWL + the PE reorder window; score errors are softmax-damped);
the P@V and Wo matmuls use float32r (rounded fp32, full PE speed at N>=512,
~1e-4 matmul precision). PSUM accumulation is fp32 throughout.
"""

import sys
import types

import numpy as np

B, L, D = 2, 4096, 512
H, HD = 8, 64
NCORES = 8
HPC = 2          # heads per core
HD2 = HPC * HD   # 128
QB = 512         # query block (free dim of S^T tiles per head)
NQB = L // QB    # 8
KC = 128         # key-position chunk (partition dim of S^T tiles)
NKC = L // KC    # 32
NDC = D // 128   # 4 contraction chunks for projections

_CACHED_NC = None


def _ensure_axon_hook():
    """Register the NTFF profile hook boot() couldn't (stub antenv lacks
    axon_hooks). Harmless when tracing is never requested."""
    try:
        from antenv.axon_hooks import get_axon_ntff_profile_hook  # noqa: F401
        return
    except ImportError:
        pass
    hook = None
    try:
        from trn_agent_boot.trn_boot import _ntff_profile_via_ctypes
        hook = _ntff_profile_via_ctypes("/opt/axon/libaxon_pjrt.so")
    except Exception:
        pass
    mod = types.ModuleType("antenv.axon_hooks")
    mod.get_axon_ntff_profile_hook = lambda: hook
    mod.set_axon_ntff_profile_hook = lambda h: None
    sys.modules["antenv.axon_hooks"] = mod


def _build_nc():
    from concourse import bacc
    import concourse.mybir as mybir
    import concourse.tile as tile

    f32 = mybir.dt.float32
    f32r = mybir.dt.float32r
    bf16 = mybir.dt.bfloat16
    AF = mybir.ActivationFunctionType

    nc = bacc.Bacc("TRN2", target_bir_lowering=False, debug=False,
                   num_devices=NCORES)

    xq = nc.dram_tensor("xq", [D, L], f32, kind="ExternalInput")
    xk = nc.dram_tensor("xk", [D, L], f32, kind="ExternalInput")
    xv = nc.dram_tensor("xv", [D, L], f32, kind="ExternalInput")
    wq = nc.dram_tensor("wq", [D, HD2], f32, kind="ExternalInput")
    wk = nc.dram_tensor("wk", [D, HD2], f32, kind="ExternalInput")
    wv = nc.dram_tensor("wv", [D, HD2], f32, kind="ExternalInput")
    wo0 = nc.dram_tensor("wo0", [HD, D], f32, kind="ExternalInput")
    wo1 = nc.dram_tensor("wo1", [HD, D], f32, kind="ExternalInput")
    bq = nc.dram_tensor("bq", [HD2, 1], f32, kind="ExternalInput")
    bk = nc.dram_tensor("bk", [HD2, 1], f32, kind="ExternalInput")
    bvb = nc.dram_tensor("bvb", [128, HD2], f32, kind="ExternalInput")
    mb = nc.dram_tensor("mb", [KC, NKC], f32, kind="ExternalInput")
    out = nc.dram_tensor("out", [L, D], f32, kind="ExternalOutput")

    with tile.TileContext(nc) as tc:
        with (
            tc.tile_pool(name="singles", bufs=1) as singles,
            tc.tile_pool(name="xload", bufs=6) as xload,
            tc.tile_pool(name="xcast", bufs=6) as xcast,
            tc.tile_pool(name="qtp", bufs=NQB) as qtp,
            tc.tile_pool(name="ptp", bufs=4) as ptp,
            tc.tile_pool(name="xtp", bufs=4) as xtp,
            tc.tile_pool(name="op", bufs=3) as op,
            tc.tile_pool(name="small", bufs=4) as small,
            tc.tile_pool(name="dscr", bufs=2, space="DRAM") as dscr,
            tc.tile_pool(name="ps_s", bufs=2, space="PSUM") as ps_sp,
            tc.tile_pool(name="ps_u", bufs=4, space="PSUM") as ps_up,
        ):
            # ---------------- constants / weights ----------------
            def load_w(name, dram):
                wf = singles.tile([128, NDC, HD2], f32, tag=name + "f")
                nc.sync.dma_start(wf[:], dram.rearrange("(c p) m -> p c m", p=128))
                wr = singles.tile([128, NDC, HD2], bf16, tag=name)
                nc.vector.tensor_copy(wr[:], wf[:])
                return wr

            wq_sb = load_w("wq", wq)
            wk_sb = load_w("wk", wk)
            wv_sb = load_w("wv", wv)

            wo0_f = singles.tile([HD, D], f32, tag="wo0f")
            wo1_f = singles.tile([HD, D], f32, tag="wo1f")
            nc.sync.dma_start(wo0_f[:], wo0[:, :])
            nc.sync.dma_start(wo1_f[:], wo1[:, :])
            wo0_sb = singles.tile([HD, D], f32r, tag="wo0")
            wo1_sb = singles.tile([HD, D], f32r, tag="wo1")
            nc.vector.tensor_copy(wo0_sb[:], wo0_f[:])
            nc.vector.tensor_copy(wo1_sb[:], wo1_f[:])

            bq_sb = singles.tile([HD2, 1], f32, tag="bq")
            bk_sb = singles.tile([HD2, 1], f32, tag="bk")
            bvb_sb = singles.tile([128, HD2], f32, tag="bvb")
            mb_sb = singles.tile([KC, NKC], f32, tag="mb")
            nc.sync.dma_start(bq_sb[:], bq[:, :])
            nc.sync.dma_start(bk_sb[:], bk[:, :])
            nc.sync.dma_start(bvb_sb[:], bvb[:, :])
            nc.sync.dma_start(mb_sb[:], mb[:, :])

            # K^T [hd2, L] bf16 and V' [kpos, chunk, hd+1] f32r per head
            kt = singles.tile([HD2, L], bf16, tag="kt")
            v0 = singles.tile([128, NKC, HD + 1], f32r, tag="v0")
            v1 = singles.tile([128, NKC, HD + 1], f32r, tag="v1")
            nc.vector.memset(v0[:, :, HD:HD + 1].bitcast(f32), 1.0)
            nc.vector.memset(v1[:, :, HD:HD + 1].bitcast(f32), 1.0)

            def load_x_block(dram, lb):
                """DMA a [128, QB] f32 tile per D-chunk and cast to bf16."""
                tiles = []
                for dc in range(NDC):
                    xf = xload.tile([128, QB], f32, tag="xl")
                    nc.sync.dma_start(
                        xf[:], dram[dc * 128:(dc + 1) * 128, lb * QB:(lb + 1) * QB])
                    xr = xcast.tile([128, QB], bf16, tag="xc")
                    nc.vector.tensor_copy(xr[:], xf[:])
                    tiles.append(xr)
                return tiles

            # ---------------- K projection ----------------
            for lb in range(NQB):
                xts = load_x_block(xk, lb)
                ps_kt = ps_sp.tile([128, QB], f32, tag="pss")
                for dc in range(NDC):
                    nc.tensor.matmul(ps_kt[:], wk_sb[:, dc, :], xts[dc][:],
                                     start=(dc == 0), stop=(dc == NDC - 1))
                nc.vector.tensor_scalar_add(
                    kt[:, lb * QB:(lb + 1) * QB], in0=ps_kt[:], scalar1=bk_sb[:])

            # ---------------- V projection ----------------
            for lb in range(NQB):
                xts = load_x_block(xv, lb)
                for j in range(4):
                    lc = lb * 4 + j
                    ps_v = ps_sp.tile([128, HD2], f32, tag="pss")
                    for dc in range(NDC):
                        nc.tensor.matmul(
                            ps_v[:], xts[dc][:, j * 128:(j + 1) * 128],
                            wv_sb[:, dc, :],
                            start=(dc == 0), stop=(dc == NDC - 1))
                    nc.vector.tensor_add(v0[:, lc, 0:HD], ps_v[:, 0:HD],
                                         bvb_sb[:, 0:HD])
                    nc.vector.tensor_add(v1[:, lc, 0:HD], ps_v[:, HD:HD2],
                                         bvb_sb[:, HD:HD2])

            # ---------------- attention + output, pipelined per q-block ----
            pending = None  # (u0, u1, qb) awaiting normalize + Wo

            def emit_norm(u0, u1, qb):
                """Normalize u tiles -> xt0/xt1 (runs on DVE/DMA, lags PE)."""
                r0 = small.tile([HD + 1, QB], f32, tag="r0")
                r1 = small.tile([HD + 1, QB], f32, tag="r1")
                nc.vector.reciprocal(r0[HD:HD + 1, :], u0[HD:HD + 1, :])
                nc.vector.reciprocal(r1[HD:HD + 1, :], u1[HD:HD + 1, :])
                rb0 = small.tile([HD, QB], f32, tag="rb0")
                rb1 = small.tile([HD, QB], f32, tag="rb1")
                scr = dscr.tile([2, QB], f32, tag="scr")
                nc.sync.dma_start(scr[0:1, :], r0[HD:HD + 1, :])
                nc.sync.dma_start(scr[1:2, :], r1[HD:HD + 1, :])
                nc.sync.dma_start(rb0[:], scr[0:1, :].to_broadcast([HD, QB]))
                nc.sync.dma_start(rb1[:], scr[1:2, :].to_broadcast([HD, QB]))
                xt0 = xtp.tile([HD, QB], f32r, tag="xt0")
                xt1 = xtp.tile([HD, QB], f32r, tag="xt1")
                nc.vector.tensor_mul(xt0[:], u0[0:HD, :], rb0[:])
                nc.vector.tensor_mul(xt1[:], u1[0:HD, :], rb1[:])
                return (xt0, xt1, qb)

            def emit_wo(xt0, xt1, qb):
                for j in range(4):
                    qs = slice(j * 128, (j + 1) * 128)
                    ps_o = ps_sp.tile([128, D], f32, tag="pss")
                    nc.tensor.matmul(ps_o[:], xt0[:, qs], wo0_sb[:],
                                     start=True, stop=False)
                    nc.tensor.matmul(ps_o[:], xt1[:, qs], wo1_sb[:],
                                     start=False, stop=True)
                    o_t = op.tile([128, D], f32, tag="ot")
                    nc.vector.tensor_copy(o_t[:], ps_o[:])
                    nc.sync.dma_start(
                        out[qb * QB + j * 128: qb * QB + (j + 1) * 128, :], o_t[:])

            for qb in range(NQB):
                # Q projection for this q-block -> qt [hd2, QB] bf16
                xts = load_x_block(xq, qb)
                ps_q = ps_sp.tile([128, QB], f32, tag="pss")
                for dc in range(NDC):
                    nc.tensor.matmul(ps_q[:], wq_sb[:, dc, :], xts[dc][:],
                                     start=(dc == 0), stop=(dc == NDC - 1))
                qt = qtp.tile([HD2, QB], bf16, tag="qt")
                nc.vector.tensor_scalar_add(qt[:], in0=ps_q[:], scalar1=bq_sb[:])

                norm_pending = None
                if pending is not None:
                    norm_pending = emit_norm(*pending)
                    pending = None

                u0 = ps_up.tile([HD + 1, QB], f32, tag="u")
                u1 = ps_up.tile([HD + 1, QB], f32, tag="u")

                def emit_pv(pt, c):
                    nc.tensor.matmul(u0[:], v0[:, c, :], pt[:, 0:QB],
                                     start=(c == 0), stop=(c == NKC - 1))
                    nc.tensor.matmul(u1[:], v1[:, c, :], pt[:, QB:2 * QB],
                                     start=(c == 0), stop=(c == NKC - 1))

                pv_q = []
                for c in range(NKC):
                    ks = slice(c * KC, (c + 1) * KC)
                    ps_s = ps_sp.tile([128, 2 * QB], f32, tag="pss")
                    nc.tensor.matmul(ps_s[:, 0:QB], kt[0:HD, ks], qt[0:HD, :],
                                     start=True, stop=True)
                    nc.tensor.matmul(ps_s[:, QB:2 * QB], kt[HD:HD2, ks],
                                     qt[HD:HD2, :], start=True, stop=True)
                    pt = ptp.tile([128, 2 * QB], f32r, tag="pt")
                    nc.scalar.activation(pt[:], ps_s[:], AF.Exp,
                                         bias=mb_sb[:, c:c + 1], scale=0.125)
                    pv_q.append((pt, c))
                    if len(pv_q) > 2:
                        emit_pv(*pv_q.pop(0))
                    if c == 8 and norm_pending is not None:
                        emit_wo(*norm_pending)
                        norm_pending = None
                for item in pv_q:
                    emit_pv(*item)
                pending = (u0, u1, qb)

            emit_wo(*emit_norm(*pending))

    nc.compile()
    return nc


def _get_nc():
    global _CACHED_NC
    if _CACHED_NC is None:
        _ensure_axon_hook()
        _CACHED_NC = _build_nc()
    return _CACHED_NC


def kernel(query, key, value, mask, Wq, bq, Wk, bk, Wv, bv, Wo, bo,
           _trace=False, _results_sink=None):
    from concourse.bass_utils import run_bass_kernel_spmd

    query = np.asarray(query, np.float32)
    key = np.asarray(key, np.float32)
    value = np.asarray(value, np.float32)
    mask = np.asarray(mask)
    Wq = np.asarray(Wq, np.float32)
    bq = np.asarray(bq, np.float32)
    Wk = np.asarray(Wk, np.float32)
    bk = np.asarray(bk, np.float32)
    Wv = np.asarray(Wv, np.float32)
    bv = np.asarray(bv, np.float32)
    Wo = np.asarray(Wo, np.float32)
    bo = np.asarray(bo, np.float32)

    nc = _get_nc()

    xqT = [np.ascontiguousarray(query[b].T) for b in range(B)]
    xkT = [np.ascontiguousarray(key[b].T) for b in range(B)]
    xvT = [np.ascontiguousarray(value[b].T) for b in range(B)]
    mbias = [
        np.ascontiguousarray(
            ((1 - mask[b].astype(np.float32)) * -1e30)
            .astype(np.float32).reshape(NKC, KC).T)
        for b in range(B)
    ]

    in_maps = []
    for core in range(NCORES):
        b = core // 4
        h0 = (core % 4) * HPC
        sl = slice(h0 * HD, (h0 + HPC) * HD)
        in_maps.append({
            "xq": xqT[b],
            "xk": xkT[b],
            "xv": xvT[b],
            "wq": np.ascontiguousarray(Wq[:, sl]),
            "wk": np.ascontiguousarray(Wk[:, sl]),
            "wv": np.ascontiguousarray(Wv[:, sl]),
            "wo0": np.ascontiguousarray(Wo[sl, :][0:HD]),
            "wo1": np.ascontiguousarray(Wo[sl, :][HD:HD2]),
            "bq": np.ascontiguousarray(bq[sl].reshape(HD2, 1)),
            "bk": np.ascontiguousarray(bk[sl].reshape(HD2, 1)),
            "bvb": np.ascontiguousarray(np.tile(bv[sl][None, :], (128, 1))),
            "mb": mbias[b],
        })

    res = run_bass_kernel_spmd(nc, in_maps, core_ids=list(range(NCORES)),
                               trace=_trace)
    if _results_sink is not None:
        _results_sink.append(res)

    final = np.empty((B, L, D), np.float32)
    for b in range(B):
        acc = res.results[4 * b]["out"].astype(np.float32).copy()
        for i in range(1, 4):
            acc += res.results[4 * b + i]["out"]
        final[b] = acc + bo[None, :]
    return final


# revision 12
# speedup vs baseline: 1.1763x; 1.0196x over previous
"""Multi-head attention (B=2, L=4096, D=512, H=8, HD=64) on 8 trn2 NeuronCores.

Sharding: data-parallel over batch (2) x tensor-parallel over head-pairs (4):
core c handles batch c//4, heads (c%4)*2 and (c%4)*2+1. Each core projects
Q/K/V for its two heads (columns of Wq/Wk/Wv), runs flash-style attention
(S^T orientation, no-max-subtraction exp since logits are small, denominator
via an appended ones-column in V), applies its rows of Wo, and returns a
partial [L, D] output. Host sums the 4 partials per batch and adds bo.

Precision: S-path matmuls (projections + scores) use bf16 operands (weight
loads hide via FWL + the PE reorder window; score errors are softmax-damped);
the P@V and Wo matmuls use float32r (rounded fp32, full PE speed at N>=512,
~1e-4 matmul precision). PSUM accumulation is fp32 throughout.
"""

import sys
import types

import numpy as np

B, L, D = 2, 4096, 512
H, HD = 8, 64
NCORES = 8
HPC = 2          # heads per core
HD2 = HPC * HD   # 128
QB = 512         # query block (free dim of S^T tiles per head)
NQB = L // QB    # 8
KC = 128         # key-position chunk (partition dim of S^T tiles)
NKC = L // KC    # 32
NDC = D // 128   # 4 contraction chunks for projections

_CACHED_NC = None


def _ensure_axon_hook():
    """Register the NTFF profile hook boot() couldn't (stub antenv lacks
    axon_hooks). Harmless when tracing is never requested."""
    try:
        from antenv.axon_hooks import get_axon_ntff_profile_hook  # noqa: F401
        return
    except ImportError:
        pass
    hook = None
    try:
        from trn_agent_boot.trn_boot import _ntff_profile_via_ctypes
        hook = _ntff_profile_via_ctypes("/opt/axon/libaxon_pjrt.so")
    except Exception:
        pass
    mod = types.ModuleType("antenv.axon_hooks")
    mod.get_axon_ntff_profile_hook = lambda: hook
    mod.set_axon_ntff_profile_hook = lambda h: None
    sys.modules["antenv.axon_hooks"] = mod


def _build_nc():
    from concourse import bacc
    import concourse.mybir as mybir
    import concourse.tile as tile

    f32 = mybir.dt.float32
    f32r = mybir.dt.float32r
    bf16 = mybir.dt.bfloat16
    AF = mybir.ActivationFunctionType

    nc = bacc.Bacc("TRN2", target_bir_lowering=False, debug=False,
                   num_devices=NCORES)

    xq = nc.dram_tensor("xq", [D, L], f32, kind="ExternalInput")
    xk = nc.dram_tensor("xk", [D, L], f32, kind="ExternalInput")
    xv = nc.dram_tensor("xv", [D, L], f32, kind="ExternalInput")
    wq = nc.dram_tensor("wq", [D, HD2], f32, kind="ExternalInput")
    wk = nc.dram_tensor("wk", [D, HD2], f32, kind="ExternalInput")
    wv = nc.dram_tensor("wv", [D, HD2], f32, kind="ExternalInput")
    wo0 = nc.dram_tensor("wo0", [HD, D], f32, kind="ExternalInput")
    wo1 = nc.dram_tensor("wo1", [HD, D], f32, kind="ExternalInput")
    bq = nc.dram_tensor("bq", [HD2, 1], f32, kind="ExternalInput")
    bk = nc.dram_tensor("bk", [HD2, 1], f32, kind="ExternalInput")
    bvb = nc.dram_tensor("bvb", [128, HD2], f32, kind="ExternalInput")
    mb = nc.dram_tensor("mb", [KC, NKC], f32, kind="ExternalInput")
    out = nc.dram_tensor("out", [L, D], f32, kind="ExternalOutput")

    with tile.TileContext(nc) as tc:
        with (
            tc.tile_pool(name="singles", bufs=1) as singles,
            tc.tile_pool(name="xload", bufs=6) as xload,
            tc.tile_pool(name="xcast", bufs=6) as xcast,
            tc.tile_pool(name="qtp", bufs=NQB) as qtp,
            tc.tile_pool(name="ptp", bufs=4) as ptp,
            tc.tile_pool(name="xtp", bufs=4) as xtp,
            tc.tile_pool(name="op", bufs=3) as op,
            tc.tile_pool(name="small", bufs=4) as small,
            tc.tile_pool(name="dscr", bufs=2, space="DRAM") as dscr,
            tc.tile_pool(name="ps_s", bufs=2, space="PSUM") as ps_sp,
            tc.tile_pool(name="ps_u", bufs=4, space="PSUM") as ps_up,
        ):
            # ---------------- constants / weights ----------------
            def load_w(name, dram):
                wf = singles.tile([128, NDC, HD2], f32, tag=name + "f")
                nc.sync.dma_start(wf[:], dram.rearrange("(c p) m -> p c m", p=128))
                wr = singles.tile([128, NDC, HD2], bf16, tag=name)
                nc.vector.tensor_copy(wr[:], wf[:])
                return wr

            wq_sb = load_w("wq", wq)
            wk_sb = load_w("wk", wk)
            wv_sb = load_w("wv", wv)

            wo0_f = singles.tile([HD, D], f32, tag="wo0f")
            wo1_f = singles.tile([HD, D], f32, tag="wo1f")
            nc.sync.dma_start(wo0_f[:], wo0[:, :])
            nc.sync.dma_start(wo1_f[:], wo1[:, :])
            wo0_sb = singles.tile([HD, D], f32r, tag="wo0")
            wo1_sb = singles.tile([HD, D], f32r, tag="wo1")
            nc.vector.tensor_copy(wo0_sb[:], wo0_f[:])
            nc.vector.tensor_copy(wo1_sb[:], wo1_f[:])

            bq_sb = singles.tile([HD2, 1], f32, tag="bq")
            bk_sb = singles.tile([HD2, 1], f32, tag="bk")
            bvb_sb = singles.tile([128, HD2], f32, tag="bvb")
            mb_sb = singles.tile([KC, NKC], f32, tag="mb")
            nc.sync.dma_start(bq_sb[:], bq[:, :])
            nc.sync.dma_start(bk_sb[:], bk[:, :])
            nc.sync.dma_start(bvb_sb[:], bvb[:, :])
            nc.sync.dma_start(mb_sb[:], mb[:, :])

            # K^T [hd2, L] bf16 (one tile per L-block for fine-grained
            # deps) and V' [kpos, hd+1] f32r per (head, kpos-chunk)
            kt_t = [singles.tile([HD2, QB], bf16, tag=f"kt{i}", name=f"kt{i}")
                    for i in range(NQB)]
            v0_t = [singles.tile([128, HD + 1], f32r, tag=f"v0_{i}", name=f"v0_{i}")
                    for i in range(NKC)]
            v1_t = [singles.tile([128, HD + 1], f32r, tag=f"v1_{i}", name=f"v1_{i}")
                    for i in range(NKC)]
            for i in range(NKC):
                nc.vector.memset(v0_t[i][:, HD:HD + 1].bitcast(f32), 1.0)
                nc.vector.memset(v1_t[i][:, HD:HD + 1].bitcast(f32), 1.0)

            def load_x_block(dram, lb, tagp="x"):
                """DMA a [128, QB] f32 tile per D-chunk and cast to bf16."""
                tiles = []
                for dc in range(NDC):
                    xf = xload.tile([128, QB], f32, tag=tagp + "l", name="xf")
                    nc.sync.dma_start(
                        xf[:], dram[dc * 128:(dc + 1) * 128, lb * QB:(lb + 1) * QB])
                    xr = xcast.tile([128, QB], bf16, tag=tagp + "c", name="xr")
                    nc.vector.tensor_copy(xr[:], xf[:])
                    tiles.append(xr)
                return tiles

            # ---------------- K projection ----------------
            for lb in range(NQB):
                xts = load_x_block(xk, lb)
                ps_kt = ps_sp.tile([128, QB], f32, tag="pss")
                for dc in range(NDC):
                    nc.tensor.matmul(ps_kt[:], wk_sb[:, dc, :], xts[dc][:],
                                     start=(dc == 0), stop=(dc == NDC - 1))
                nc.vector.tensor_scalar_add(
                    kt_t[lb][:], in0=ps_kt[:], scalar1=bk_sb[:])

            # prefetch the first attention q-block's inputs ahead of V
            q0_tiles = load_x_block(xq, 0, tagp='q0')

            # ---------------- V projection ----------------
            for lb in range(NQB):
                xts = load_x_block(xv, lb)
                for j in range(4):
                    lc = lb * 4 + j
                    ps_v = ps_sp.tile([128, HD2], f32, tag="pss")
                    for dc in range(NDC):
                        nc.tensor.matmul(
                            ps_v[:], xts[dc][:, j * 128:(j + 1) * 128],
                            wv_sb[:, dc, :],
                            start=(dc == 0), stop=(dc == NDC - 1))
                    nc.vector.tensor_add(v0_t[lc][:, 0:HD], ps_v[:, 0:HD],
                                         bvb_sb[:, 0:HD])
                    nc.vector.tensor_add(v1_t[lc][:, 0:HD], ps_v[:, HD:HD2],
                                         bvb_sb[:, HD:HD2])

            # ---------------- attention + output, pipelined per q-block ----
            pending = None  # (u0, u1, qb) awaiting normalize + Wo

            def emit_norm(u0, u1, qb):
                """Normalize u tiles -> xt0/xt1 (runs on DVE/DMA, lags PE)."""
                r0 = small.tile([HD + 1, QB], f32, tag="r0")
                r1 = small.tile([HD + 1, QB], f32, tag="r1")
                nc.vector.reciprocal(r0[HD:HD + 1, :], u0[HD:HD + 1, :])
                nc.vector.reciprocal(r1[HD:HD + 1, :], u1[HD:HD + 1, :])
                rb0 = small.tile([HD, QB], f32, tag="rb0")
                rb1 = small.tile([HD, QB], f32, tag="rb1")
                scr = dscr.tile([2, QB], f32, tag="scr")
                nc.sync.dma_start(scr[0:1, :], r0[HD:HD + 1, :])
                nc.sync.dma_start(scr[1:2, :], r1[HD:HD + 1, :])
                nc.sync.dma_start(rb0[:], scr[0:1, :].to_broadcast([HD, QB]))
                nc.sync.dma_start(rb1[:], scr[1:2, :].to_broadcast([HD, QB]))
                xt0 = xtp.tile([HD, QB], f32r, tag="xt0")
                xt1 = xtp.tile([HD, QB], f32r, tag="xt1")
                nc.vector.tensor_mul(xt0[:], u0[0:HD, :], rb0[:])
                nc.vector.tensor_mul(xt1[:], u1[0:HD, :], rb1[:])
                return (xt0, xt1, qb)

            def emit_wo(xt0, xt1, qb):
                for j in range(4):
                    qs = slice(j * 128, (j + 1) * 128)
                    ps_o = ps_sp.tile([128, D], f32, tag="pss")
                    nc.tensor.matmul(ps_o[:], xt0[:, qs], wo0_sb[:],
                                     start=True, stop=False)
                    nc.tensor.matmul(ps_o[:], xt1[:, qs], wo1_sb[:],
                                     start=False, stop=True)
                    o_t = op.tile([128, D], f32, tag="ot")
                    nc.vector.tensor_copy(o_t[:], ps_o[:])
                    nc.sync.dma_start(
                        out[qb * QB + j * 128: qb * QB + (j + 1) * 128, :], o_t[:])

            for qb in range(NQB):
                # Q projection for this q-block -> qt [hd2, QB] bf16
                xts = q0_tiles if qb == 0 else load_x_block(xq, qb)
                ps_q = ps_sp.tile([128, QB], f32, tag="pss")
                for dc in range(NDC):
                    nc.tensor.matmul(ps_q[:], wq_sb[:, dc, :], xts[dc][:],
                                     start=(dc == 0), stop=(dc == NDC - 1))
                qt = qtp.tile([HD2, QB], bf16, tag="qt")
                nc.vector.tensor_scalar_add(qt[:], in0=ps_q[:], scalar1=bq_sb[:])

                u0 = ps_up.tile([HD + 1, QB], f32, tag="u")
                u1 = ps_up.tile([HD + 1, QB], f32, tag="u")

                def emit_pv(pt, c):
                    nc.tensor.matmul(u0[:], v0_t[c][:], pt[:, 0:QB],
                                     start=(c == 0), stop=(c == NKC - 1))
                    nc.tensor.matmul(u1[:], v1_t[c][:], pt[:, QB:2 * QB],
                                     start=(c == 0), stop=(c == NKC - 1))

                pv_q = []
                norm_pending = None
                for c in range(NKC):
                    kb, ko = c // 4, (c % 4) * KC
                    ks = slice(ko, ko + KC)
                    ps_s = ps_sp.tile([128, 2 * QB], f32, tag="pss")
                    nc.tensor.matmul(ps_s[:, 0:QB], kt_t[kb][0:HD, ks],
                                     qt[0:HD, :], start=True, stop=True)
                    nc.tensor.matmul(ps_s[:, QB:2 * QB], kt_t[kb][HD:HD2, ks],
                                     qt[HD:HD2, :], start=True, stop=True)
                    pt = ptp.tile([128, 2 * QB], f32r, tag="pt")
                    nc.scalar.activation(pt[:], ps_s[:], AF.Exp,
                                         bias=mb_sb[:, c:c + 1], scale=0.125)
                    pv_q.append((pt, c))
                    if len(pv_q) > 2:
                        emit_pv(*pv_q.pop(0))
                    if c == 4 and pending is not None:
                        norm_pending = emit_norm(*pending)
                        pending = None
                    if c == 12 and norm_pending is not None:
                        emit_wo(*norm_pending)
                        norm_pending = None
                for item in pv_q:
                    emit_pv(*item)
                pending = (u0, u1, qb)

            emit_wo(*emit_norm(*pending))

    nc.compile()
    return nc


def _get_nc():
    global _CACHED_NC
    if _CACHED_NC is None:
        _ensure_axon_hook()
        _CACHED_NC = _build_nc()
    return _CACHED_NC


def kernel(query, key, value, mask, Wq, bq, Wk, bk, Wv, bv, Wo, bo,
           _trace=False, _results_sink=None):
    from concourse.bass_utils import run_bass_kernel_spmd

    query = np.asarray(query, np.float32)
    key = np.asarray(key, np.float32)
    value = np.asarray(value, np.float32)
    mask = np.asarray(mask)
    Wq = np.asarray(Wq, np.float32)
    bq = np.asarray(bq, np.float32)
    Wk = np.asarray(Wk, np.float32)
    bk = np.asarray(bk, np.float32)
    Wv = np.asarray(Wv, np.float32)
    bv = np.asarray(bv, np.float32)
    Wo = np.asarray(Wo, np.float32)
    bo = np.asarray(bo, np.float32)

    nc = _get_nc()

    xqT = [np.ascontiguousarray(query[b].T) for b in range(B)]
    xkT = [np.ascontiguousarray(key[b].T) for b in range(B)]
    xvT = [np.ascontiguousarray(value[b].T) for b in range(B)]
    mbias = [
        np.ascontiguousarray(
            ((1 - mask[b].astype(np.float32)) * -1e30)
            .astype(np.float32).reshape(NKC, KC).T)
        for b in range(B)
    ]

    in_maps = []
    for core in range(NCORES):
        b = core // 4
        h0 = (core % 4) * HPC
        sl = slice(h0 * HD, (h0 + HPC) * HD)
        in_maps.append({
            "xq": xqT[b],
            "xk": xkT[b],
            "xv": xvT[b],
            "wq": np.ascontiguousarray(Wq[:, sl]),
            "wk": np.ascontiguousarray(Wk[:, sl]),
            "wv": np.ascontiguousarray(Wv[:, sl]),
            "wo0": np.ascontiguousarray(Wo[sl, :][0:HD]),
            "wo1": np.ascontiguousarray(Wo[sl, :][HD:HD2]),
            "bq": np.ascontiguousarray(bq[sl].reshape(HD2, 1)),
            "bk": np.ascontiguousarray(bk[sl].reshape(HD2, 1)),
            "bvb": np.ascontiguousarray(np.tile(bv[sl][None, :], (128, 1))),
            "mb": mbias[b],
        })

    res = run_bass_kernel_spmd(nc, in_maps, core_ids=list(range(NCORES)),
                               trace=_trace)
    if _results_sink is not None:
        _results_sink.append(res)

    final = np.empty((B, L, D), np.float32)
    for b in range(B):
        acc = res.results[4 * b]["out"].astype(np.float32).copy()
        for i in range(1, 4):
            acc += res.results[4 * b + i]["out"]
        final[b] = acc + bo[None, :]
    return final


# revision 13
# speedup vs baseline: 1.2787x; 1.0870x over previous
"""Multi-head attention (B=2, L=4096, D=512, H=8, HD=64) on 8 trn2 NeuronCores.

Sharding: data-parallel over batch (2) x tensor-parallel over head-pairs (4):
core c handles batch c//4, heads (c%4)*2 and (c%4)*2+1. Each core projects
Q/K/V for its two heads (columns of Wq/Wk/Wv), runs flash-style attention
(S^T orientation, no-max-subtraction exp since logits are small, denominator
via an appended ones-column in V), applies its rows of Wo, and returns a
partial [L, D] output. Host sums the 4 partials per batch and adds bo.

Precision: S-path matmuls (projections + scores) use bf16 operands (weight
loads hide via FWL + the PE reorder window; score errors are softmax-damped);
the P@V and Wo matmuls use float32r (rounded fp32, full PE speed at N>=512,
~1e-4 matmul precision). PSUM accumulation is fp32 throughout.
"""

import sys
import types

import numpy as np

B, L, D = 2, 4096, 512
H, HD = 8, 64
NCORES = 8
HPC = 2          # heads per core
HD2 = HPC * HD   # 128
QB = 512         # query block (free dim of S^T tiles per head)
NQB = L // QB    # 8
KC = 128         # key-position chunk (partition dim of S^T tiles)
NKC = L // KC    # 32
NDC = D // 128   # 4 contraction chunks for projections

_CACHED_NC = None


def _ensure_axon_hook():
    """Register the NTFF profile hook boot() couldn't (stub antenv lacks
    axon_hooks). Harmless when tracing is never requested."""
    try:
        from antenv.axon_hooks import get_axon_ntff_profile_hook  # noqa: F401
        return
    except ImportError:
        pass
    hook = None
    try:
        from trn_agent_boot.trn_boot import _ntff_profile_via_ctypes
        hook = _ntff_profile_via_ctypes("/opt/axon/libaxon_pjrt.so")
    except Exception:
        pass
    mod = types.ModuleType("antenv.axon_hooks")
    mod.get_axon_ntff_profile_hook = lambda: hook
    mod.set_axon_ntff_profile_hook = lambda h: None
    sys.modules["antenv.axon_hooks"] = mod


def _build_nc():
    from concourse import bacc
    import concourse.mybir as mybir
    import concourse.tile as tile

    f32 = mybir.dt.float32
    f32r = mybir.dt.float32r
    bf16 = mybir.dt.bfloat16
    AF = mybir.ActivationFunctionType

    nc = bacc.Bacc("TRN2", target_bir_lowering=False, debug=False,
                   num_devices=NCORES)

    xq = nc.dram_tensor("xq", [D, L], f32, kind="ExternalInput")
    xk = nc.dram_tensor("xk", [D, L], f32, kind="ExternalInput")
    xv = nc.dram_tensor("xv", [D, L], f32, kind="ExternalInput")
    wq = nc.dram_tensor("wq", [D, HD2], f32, kind="ExternalInput")
    wk = nc.dram_tensor("wk", [D, HD2], f32, kind="ExternalInput")
    wv = nc.dram_tensor("wv", [D, HD2], f32, kind="ExternalInput")
    wo0 = nc.dram_tensor("wo0", [HD, D], f32, kind="ExternalInput")
    wo1 = nc.dram_tensor("wo1", [HD, D], f32, kind="ExternalInput")
    bq = nc.dram_tensor("bq", [HD2, 1], f32, kind="ExternalInput")
    bk = nc.dram_tensor("bk", [HD2, 1], f32, kind="ExternalInput")
    bvb = nc.dram_tensor("bvb", [128, HD2], f32, kind="ExternalInput")
    mb = nc.dram_tensor("mb", [KC, NKC], f32, kind="ExternalInput")
    out = nc.dram_tensor("out", [L, D], f32, kind="ExternalOutput")

    with tile.TileContext(nc) as tc:
        with (
            tc.tile_pool(name="singles", bufs=1) as singles,
            tc.tile_pool(name="xload", bufs=10) as xload,
            tc.tile_pool(name="xcast", bufs=10) as xcast,
            tc.tile_pool(name="qtp", bufs=NQB) as qtp,
            tc.tile_pool(name="ptp", bufs=4) as ptp,
            tc.tile_pool(name="xtp", bufs=4) as xtp,
            tc.tile_pool(name="op", bufs=3) as op,
            tc.tile_pool(name="small", bufs=4) as small,
            tc.tile_pool(name="dscr", bufs=2, space="DRAM") as dscr,
            tc.tile_pool(name="ps_s", bufs=2, space="PSUM") as ps_sp,
            tc.tile_pool(name="ps_u", bufs=4, space="PSUM") as ps_up,
        ):
            # ---------------- constants / weights ----------------
            def load_w(name, dram):
                wf = singles.tile([128, NDC, HD2], f32, tag=name + "f")
                nc.sync.dma_start(wf[:], dram.rearrange("(c p) m -> p c m", p=128))
                wr = singles.tile([128, NDC, HD2], bf16, tag=name)
                nc.vector.tensor_copy(wr[:], wf[:])
                return wr

            wq_sb = load_w("wq", wq)
            wk_sb = load_w("wk", wk)
            wv_sb = load_w("wv", wv)

            wo0_f = singles.tile([HD, D], f32, tag="wo0f")
            wo1_f = singles.tile([HD, D], f32, tag="wo1f")
            nc.sync.dma_start(wo0_f[:], wo0[:, :])
            nc.sync.dma_start(wo1_f[:], wo1[:, :])
            wo0_sb = singles.tile([HD, D], f32r, tag="wo0")
            wo1_sb = singles.tile([HD, D], f32r, tag="wo1")
            nc.vector.tensor_copy(wo0_sb[:], wo0_f[:])
            nc.vector.tensor_copy(wo1_sb[:], wo1_f[:])

            bq_sb = singles.tile([HD2, 1], f32, tag="bq")
            bk_sb = singles.tile([HD2, 1], f32, tag="bk")
            bvb_sb = singles.tile([128, HD2], f32, tag="bvb")
            mb_sb = singles.tile([KC, NKC], f32, tag="mb")
            nc.sync.dma_start(bq_sb[:], bq[:, :])
            nc.sync.dma_start(bk_sb[:], bk[:, :])
            nc.sync.dma_start(bvb_sb[:], bvb[:, :])
            nc.sync.dma_start(mb_sb[:], mb[:, :])

            # K^T [hd2, L] bf16 (one tile per L-block for fine-grained
            # deps) and V' [kpos, hd+1] f32r per (head, kpos-chunk)
            kt_t = [singles.tile([HD2, QB], bf16, tag=f"kt{i}", name=f"kt{i}")
                    for i in range(NQB)]
            v0_t = [singles.tile([128, HD + 1], f32r, tag=f"v0_{i}", name=f"v0_{i}")
                    for i in range(NKC)]
            v1_t = [singles.tile([128, HD + 1], f32r, tag=f"v1_{i}", name=f"v1_{i}")
                    for i in range(NKC)]
            for i in range(NKC):
                nc.vector.memset(v0_t[i][:, HD:HD + 1].bitcast(f32), 1.0)
                nc.vector.memset(v1_t[i][:, HD:HD + 1].bitcast(f32), 1.0)

            def load_x_block(dram, lb, tagp="x"):
                """DMA a [128, QB] f32 tile per D-chunk and cast to bf16."""
                tiles = []
                for dc in range(NDC):
                    xf = xload.tile([128, QB], f32, tag=tagp + "l", name="xf")
                    nc.sync.dma_start(
                        xf[:], dram[dc * 128:(dc + 1) * 128, lb * QB:(lb + 1) * QB])
                    xr = xcast.tile([128, QB], bf16, tag=tagp + "c", name="xr")
                    nc.vector.tensor_copy(xr[:], xf[:])
                    tiles.append(xr)
                return tiles

            def emit_kproj(lb, xts=None):
                if xts is None:
                    xts = load_x_block(xk, lb)
                ps_kt = ps_sp.tile([128, QB], f32, tag="pss", name="ps_kt")
                for dc in range(NDC):
                    nc.tensor.matmul(ps_kt[:], wk_sb[:, dc, :], xts[dc][:],
                                     start=(dc == 0), stop=(dc == NDC - 1))
                nc.vector.tensor_scalar_add(
                    kt_t[lb][:], in0=ps_kt[:], scalar1=bk_sb[:])

            def emit_vproj_lc(xts, lc):
                j = lc % 4
                ps_v = ps_sp.tile([128, HD2], f32, tag="pss", name="ps_v")
                for dc in range(NDC):
                    nc.tensor.matmul(
                        ps_v[:], xts[dc][:, j * 128:(j + 1) * 128],
                        wv_sb[:, dc, :],
                        start=(dc == 0), stop=(dc == NDC - 1))
                nc.vector.tensor_add(v0_t[lc][:, 0:HD], ps_v[:, 0:HD],
                                     bvb_sb[:, 0:HD])
                nc.vector.tensor_add(v1_t[lc][:, 0:HD], ps_v[:, HD:HD2],
                                     bvb_sb[:, HD:HD2])

            # startup: first q-block inputs + first K block; the rest of the
            # K/V projections are interleaved into q-block 0's chunk loop so
            # attention starts as soon as kt[0]/qt[0] land (~7us).
            q0_tiles = load_x_block(xq, 0, tagp='q0')
            emit_kproj(0)

            # ---------------- attention + output, pipelined per q-block ----
            pending = None  # (u0, u1, qb) awaiting normalize + Wo

            def emit_norm(u0, u1, qb):
                """Normalize u tiles -> xt0/xt1 (runs on DVE/DMA, lags PE)."""
                r0 = small.tile([HD + 1, QB], f32, tag="r0")
                r1 = small.tile([HD + 1, QB], f32, tag="r1")
                nc.vector.reciprocal(r0[HD:HD + 1, :], u0[HD:HD + 1, :])
                nc.vector.reciprocal(r1[HD:HD + 1, :], u1[HD:HD + 1, :])
                rb0 = small.tile([HD, QB], f32, tag="rb0")
                rb1 = small.tile([HD, QB], f32, tag="rb1")
                scr = dscr.tile([2, QB], f32, tag="scr")
                nc.sync.dma_start(scr[0:1, :], r0[HD:HD + 1, :])
                nc.sync.dma_start(scr[1:2, :], r1[HD:HD + 1, :])
                nc.sync.dma_start(rb0[:], scr[0:1, :].to_broadcast([HD, QB]))
                nc.sync.dma_start(rb1[:], scr[1:2, :].to_broadcast([HD, QB]))
                xt0 = xtp.tile([HD, QB], f32r, tag="xt0")
                xt1 = xtp.tile([HD, QB], f32r, tag="xt1")
                nc.vector.tensor_mul(xt0[:], u0[0:HD, :], rb0[:])
                nc.vector.tensor_mul(xt1[:], u1[0:HD, :], rb1[:])
                return (xt0, xt1, qb)

            def emit_wo(xt0, xt1, qb):
                for j in range(4):
                    qs = slice(j * 128, (j + 1) * 128)
                    ps_o = ps_sp.tile([128, D], f32, tag="pss")
                    nc.tensor.matmul(ps_o[:], xt0[:, qs], wo0_sb[:],
                                     start=True, stop=False)
                    nc.tensor.matmul(ps_o[:], xt1[:, qs], wo1_sb[:],
                                     start=False, stop=True)
                    o_t = op.tile([128, D], f32, tag="ot")
                    nc.vector.tensor_copy(o_t[:], ps_o[:])
                    nc.sync.dma_start(
                        out[qb * QB + j * 128: qb * QB + (j + 1) * 128, :], o_t[:])

            vx_tiles = None
            for qb in range(NQB):
                # Q projection for this q-block -> qt [hd2, QB] bf16
                xts = q0_tiles if qb == 0 else load_x_block(xq, qb)
                ps_q = ps_sp.tile([128, QB], f32, tag="pss")
                for dc in range(NDC):
                    nc.tensor.matmul(ps_q[:], wq_sb[:, dc, :], xts[dc][:],
                                     start=(dc == 0), stop=(dc == NDC - 1))
                qt = qtp.tile([HD2, QB], bf16, tag="qt")
                nc.vector.tensor_scalar_add(qt[:], in0=ps_q[:], scalar1=bq_sb[:])

                u0 = ps_up.tile([HD + 1, QB], f32, tag="u")
                u1 = ps_up.tile([HD + 1, QB], f32, tag="u")

                def emit_pv(pt, c):
                    nc.tensor.matmul(u0[:], v0_t[c][:], pt[:, 0:QB],
                                     start=(c == 0), stop=(c == NKC - 1))
                    nc.tensor.matmul(u1[:], v1_t[c][:], pt[:, QB:2 * QB],
                                     start=(c == 0), stop=(c == NKC - 1))

                pv_q = []
                norm_pending = None
                for c in range(NKC):
                    if qb == 0:
                        if c % 4 == 0:
                            if c < NKC - 4:
                                emit_kproj(c // 4 + 1)
                            vx_tiles = load_x_block(xv, c // 4)
                    kb, ko = c // 4, (c % 4) * KC
                    ks = slice(ko, ko + KC)
                    ps_s = ps_sp.tile([128, 2 * QB], f32, tag="pss")
                    nc.tensor.matmul(ps_s[:, 0:QB], kt_t[kb][0:HD, ks],
                                     qt[0:HD, :], start=True, stop=True)
                    nc.tensor.matmul(ps_s[:, QB:2 * QB], kt_t[kb][HD:HD2, ks],
                                     qt[HD:HD2, :], start=True, stop=True)
                    pt = ptp.tile([128, 2 * QB], f32r, tag="pt")
                    nc.scalar.activation(pt[:], ps_s[:], AF.Exp,
                                         bias=mb_sb[:, c:c + 1], scale=0.125)
                    if qb == 0:
                        emit_vproj_lc(vx_tiles, c)
                    pv_q.append((pt, c))
                    if len(pv_q) > 2:
                        emit_pv(*pv_q.pop(0))
                    if c == 4 and pending is not None:
                        norm_pending = emit_norm(*pending)
                        pending = None
                    if c == 12 and norm_pending is not None:
                        emit_wo(*norm_pending)
                        norm_pending = None
                for item in pv_q:
                    emit_pv(*item)
                pending = (u0, u1, qb)

            emit_wo(*emit_norm(*pending))

    nc.compile()
    return nc


def _get_nc():
    global _CACHED_NC
    if _CACHED_NC is None:
        _ensure_axon_hook()
        _CACHED_NC = _build_nc()
    return _CACHED_NC


def kernel(query, key, value, mask, Wq, bq, Wk, bk, Wv, bv, Wo, bo,
           _trace=False, _results_sink=None):
    from concourse.bass_utils import run_bass_kernel_spmd

    query = np.asarray(query, np.float32)
    key = np.asarray(key, np.float32)
    value = np.asarray(value, np.float32)
    mask = np.asarray(mask)
    Wq = np.asarray(Wq, np.float32)
    bq = np.asarray(bq, np.float32)
    Wk = np.asarray(Wk, np.float32)
    bk = np.asarray(bk, np.float32)
    Wv = np.asarray(Wv, np.float32)
    bv = np.asarray(bv, np.float32)
    Wo = np.asarray(Wo, np.float32)
    bo = np.asarray(bo, np.float32)

    nc = _get_nc()

    xqT = [np.ascontiguousarray(query[b].T) for b in range(B)]
    xkT = [np.ascontiguousarray(key[b].T) for b in range(B)]
    xvT = [np.ascontiguousarray(value[b].T) for b in range(B)]
    mbias = [
        np.ascontiguousarray(
            ((1 - mask[b].astype(np.float32)) * -1e30)
            .astype(np.float32).reshape(NKC, KC).T)
        for b in range(B)
    ]

    in_maps = []
    for core in range(NCORES):
        b = core // 4
        h0 = (core % 4) * HPC
        sl = slice(h0 * HD, (h0 + HPC) * HD)
        in_maps.append({
            "xq": xqT[b],
            "xk": xkT[b],
            "xv": xvT[b],
            "wq": np.ascontiguousarray(Wq[:, sl]),
            "wk": np.ascontiguousarray(Wk[:, sl]),
            "wv": np.ascontiguousarray(Wv[:, sl]),
            "wo0": np.ascontiguousarray(Wo[sl, :][0:HD]),
            "wo1": np.ascontiguousarray(Wo[sl, :][HD:HD2]),
            "bq": np.ascontiguousarray(bq[sl].reshape(HD2, 1)),
            "bk": np.ascontiguousarray(bk[sl].reshape(HD2, 1)),
            "bvb": np.ascontiguousarray(np.tile(bv[sl][None, :], (128, 1))),
            "mb": mbias[b],
        })

    res = run_bass_kernel_spmd(nc, in_maps, core_ids=list(range(NCORES)),
                               trace=_trace)
    if _results_sink is not None:
        _results_sink.append(res)

    final = np.empty((B, L, D), np.float32)
    for b in range(B):
        acc = res.results[4 * b]["out"].astype(np.float32).copy()
        for i in range(1, 4):
            acc += res.results[4 * b + i]["out"]
        final[b] = acc + bo[None, :]
    return final
